# revision 9
# baseline (speedup 1.0000x reference)
"""Trainium2 Bass kernel for nn_ExpandEvecs.

Computes, for evecs [B=4, C=1, N=1024, K=16]:
    outers[b,k,i,j] = evecs[b,0,i,k] * evecs[b,0,j,k]
    cube = cumsum(outers, axis=k)  ->  [B, K, N, N]
i.e. cube[b,l] = V[:, :l+1] @ V[:, :l+1]^T  (Gram expansion per level).

Sharding: 8 cores = 4 batches x 2 row-halves. Core c (b=c//2, h=c%2)
computes all 16 levels for its 512-row half of batch b. No inter-core
communication. The output is stored as bf16 (16 MiB/core) and upcast to
f32 on the host during unsharding; bf16 rounding of the cube costs
~2.4e-3 max rel err vs the 2e-2 gate.

Engine split (per core):
  - EVEN levels ("anchors") on the PE: one bf16 matmul per output tile
    using the A/B split trick (V = A + B, A=bf16(V), B=bf16(V-A);
    lhsT/rhs partition-interleaved so AA^T+AB^T+BA^T comes out of a
    single matmul with contraction 3*(l+1)). The PE is output-column
    bound (~426 ns per 512-col matmul at the cold 1.2 GHz clock), so
    only half the levels go through it.
  - ODD levels derived on the Vector engine from the cumsum recurrence
    cube_l = cube_{l-1} + a_l a_l^T as ONE fused op per quarter:
    scalar_tensor_tensor: out = (rep_l * vcol) + st_prev, where rep_l
    is the a_l row replicated across partitions (built by GpSimd
    partition_broadcast) and vcol is a host-prepared transposed column.
  - Scalar (ACT) engine does the PSUM->SBUF bf16 cast copies of anchors.
  - 16 SDMA engines stream the 16 MiB of stores (~47 us at the
    ~358 GB/s HBM-per-core limit) -- the roofline this schedule targets.

Row-pair interleave: partition p of a store tile holds DRAM rows
4p..4p+3, so each partition's store run is 8 KiB contiguous.
"""

import numpy as np
import ml_dtypes

import concourse.mybir as mybir
from concourse import bacc, bass
from concourse.tile import TileContext
from concourse.bass_utils import run_bass_kernel_spmd

B, C, N, K = 4, 1, 1024, 16
NCORES = 8
HALF = N // 2          # rows per core
K3 = 3 * K             # stacked contraction partitions

F32 = mybir.dt.float32
BF16 = mybir.dt.bfloat16
BF16_NP = ml_dtypes.bfloat16

_nc_cache = None


def _build():
    nc = bacc.Bacc(None, target_bir_lowering=False)
    t_d = nc.declare_dram_parameter("t", [K3, N], BF16, isOutput=False)
    tl_d = nc.declare_dram_parameter("tl", [K3, HALF], BF16, isOutput=False)
    # ti[p, r, k] = a_k[4p + r] (this core's rows, interleaved order)
    ti_d = nc.declare_dram_parameter("ti", [128, 4, K], BF16, isOutput=False)
    out_d = nc.declare_dram_parameter("out", [K, HALF, N], BF16, isOutput=True)

    mult, add = mybir.AluOpType.mult, mybir.AluOpType.add

    with TileContext(nc) as tc:
        with (
            tc.tile_pool(name="vpool", bufs=1) as vpool,
            tc.tile_pool(name="stage", bufs=6) as stage,
            tc.tile_pool(name="psum", bufs=4, space=bass.MemorySpace.PSUM) as psum,
        ):
            t = vpool.tile([K3, N], BF16)
            tl = vpool.tile([K3, HALF], BF16)
            t0 = vpool.tile([6, N], BF16)
            tl0 = vpool.tile([6, HALF], BF16)
            ti = vpool.tile([128, 4, K], BF16)
            # single-partition staging rows for the broadcasts (the gpsimd
            # ISA op requires its input AP to start at partition 0)
            arow = {
                l: vpool.tile([1, N], BF16, name=f"arow{l}")
                for l in range(1, K, 2)
            }
            reps = {
                l: vpool.tile([128, N], BF16, name=f"rep{l}")
                for l in range(1, K, 2)
            }
            # tiny level-0/1 slices first (unblocks the first matmuls
            # earlier), then the full operands; two HWDGE rings in parallel
            nc.sync.dma_start(out=tl0[:], in_=tl_d[:6, :])
            nc.scalar.dma_start(out=t0[:], in_=t_d[:6, :])
            for l in range(1, K, 2):
                nc.scalar.dma_start(out=arow[l][:], in_=t_d[3 * l:3 * l + 1, :])
            nc.sync.dma_start(out=ti[:], in_=ti_d[:])
            nc.scalar.dma_start(out=t[:], in_=t_d[:])
            nc.sync.dma_start(out=tl[:], in_=tl_d[:])

            # rep_l[p, j] = a_l[j]: GpSimd broadcasts the a_l row across
            # all 128 partitions for the derived-level outer products.
            for l in range(1, K, 2):
                nc.gpsimd.partition_broadcast(reps[l][:], arow[l][:])

            # row-pair interleave: partition p of a store tile holds DRAM
            # rows 4p..4p+3 -> 8 KiB contiguous runs per partition.
            tlv = tl.rearrange("k (m r) -> k m r", m=128, r=4)
            tlv0 = tl0.rearrange("k (m r) -> k m r", m=128, r=4)

            for l in range(0, K, 2):
                kk = 3 * (l + 1)
                lhs_v, rhs_t = (tlv0, t0) if l == 0 else (tlv, t)
                out_v = out_d[l].rearrange("(p r) f -> p r f", p=128)
                st = stage.tile([128, 4, N], BF16, tag="st")
                for r in range(4):
                    ps = psum.tile([128, N], F32, tag="ps")
                    for j in range(2):
                        nc.tensor.matmul(
                            ps[:, j * 512:(j + 1) * 512],
                            lhsT=lhs_v[:kk, :, r],
                            rhs=rhs_t[:kk, j * 512:(j + 1) * 512],
                            start=True,
                            stop=True,
                        )
                    if l == 0:
                        # ramp: split the copy across both engines and
                        # store per-quarter to start the DMA stream early
                        nc.vector.tensor_copy(st[:, r, :512], ps[:, :512])
                        nc.scalar.copy(st[:, r, 512:], ps[:, 512:])
                        nc.sync.dma_start(out=out_v[:, r, :], in_=st[:, r, :])
                    else:
                        nc.scalar.copy(st[:, r, :], ps[:])
                if l > 0:
                    nc.sync.dma_start(out=out_v, in_=st[:, :, :])

                # derived odd level l+1: cube_{l+1} = cube_l + a_{l+1} outer
                ld = l + 1
                out_dv = out_d[ld].rearrange("(p r) f -> p r f", p=128)
                sd = stage.tile([128, 4, N], BF16, tag="st")
                for r in range(4):
                    nc.vector.scalar_tensor_tensor(
                        out=sd[:, r, :],
                        in0=reps[ld][:],
                        scalar=ti[:, r, ld:ld + 1],
                        in1=st[:, r, :],
                        op0=mult,
                        op1=add,
                    )
                    if l == 0:
                        nc.sync.dma_start(out=out_dv[:, r, :], in_=sd[:, r, :])
                if l > 0:
                    nc.sync.dma_start(out=out_dv, in_=sd[:, :, :])

    nc.compile()
    return nc


def _get_nc():
    global _nc_cache
    if _nc_cache is None:
        _nc_cache = _build()
    return _nc_cache


def _prepare_in_maps(evecs: np.ndarray) -> list[dict]:
    in_maps = []
    for c in range(NCORES):
        b, h = divmod(c, 2)
        vt = np.ascontiguousarray(evecs[b, 0].T, dtype=np.float32)  # [K, N]
        a32 = vt.astype(BF16_NP).astype(np.float32)
        a = a32.astype(BF16_NP)                       # hi part
        bb = (vt - a32).astype(BF16_NP)               # lo part
        t = np.empty((K3, N), dtype=BF16_NP)
        t[0::3] = a
        t[1::3] = bb
        t[2::3] = a
        sl = slice(h * HALF, (h + 1) * HALF)
        tl = np.empty((K3, HALF), dtype=BF16_NP)
        tl[0::3] = a[:, sl]
        tl[1::3] = a[:, sl]
        tl[2::3] = bb[:, sl]
        # ti[p, r, k] = a_k[h*HALF + 4p + r]
        ti = np.ascontiguousarray(
            a[:, sl].T.reshape(128, 4, K, order="C")
        ).astype(BF16_NP)
        in_maps.append({"t": t, "tl": tl, "ti": ti})
    return in_maps


def _assemble(results: list[dict]) -> np.ndarray:
    out = np.empty((B, K, N, N), dtype=np.float32)
    for c in range(NCORES):
        b, h = divmod(c, 2)
        out[b, :, h * HALF:(h + 1) * HALF, :] = results[c]["out"].astype(
            np.float32
        )
    return out.reshape(B, K * C, N, N)


def kernel(evecs) -> np.ndarray:
    evecs = np.asarray(evecs, dtype=np.float32)
    assert evecs.shape == (B, C, N, K), evecs.shape
    nc = _get_nc()
    in_maps = _prepare_in_maps(evecs)
    last_err = None
    for _attempt in range(3):
        try:
            r = run_bass_kernel_spmd(nc, in_maps, list(range(NCORES)))
            return _assemble(r.results)
        except Exception as e:  # transient NRT/device hiccups: retry
            last_err = e
    raise last_err


# revision 15
# speedup vs baseline: 1.2960x; 1.2960x over previous
"""Trainium2 Bass kernel for nn_ExpandEvecs.

Computes, for evecs [B=4, C=1, N=1024, K=16]:
    outers[b,k,i,j] = evecs[b,0,i,k] * evecs[b,0,j,k]
    cube = cumsum(outers, axis=k)  ->  [B, K, N, N]
i.e. cube[b,l] = V[:, :l+1] @ V[:, :l+1]^T  (Gram expansion per level).

Every level is SYMMETRIC, so the device only computes the upper
block-triangle (56% of the elements; diagonal 128-blocks in full) and
the host mirrors the strictly-lower blocks during unsharding. Output
is stored bf16 and upcast on the host (2.4e-3 max rel err vs the 2e-2
gate). Per-core HBM stores drop to 9 MiB (~26 us at the ~358 GB/s
HBM-per-core limit), PE columns and PSUM->SBUF copy work drop by the
same 2.3x vs the full-matrix version.

Sharding: 8 cores = 4 batches x 2 triangle-halves. The upper
block-triangle of each [1024,1024] level splits into six 128-row
pieces per core with IDENTICAL shapes on both cores (SPMD-safe):
sizes (512,512,512,384,256,128) columns. Piece p of a core is
(block-row i_p, cols c0_p:c1_p); the host knows the same table.

Per core, per level: 6 bf16 matmuls (one per piece, 2304 PE columns
total) using the A/B split trick (V = A + B with A = bf16(V),
B = bf16(V-A); lhsT/rhs partition-interleaved so AA^T+AB^T+BA^T comes
out of one matmul with contraction 3*(l+1); the dropped BB^T term is
~2^-18 relative). Pieces pack pairwise into three PSUM tiles
([128,1024], [128,896], [128,384]) so each level needs only three
PSUM->SBUF bf16 cast copies, alternating Vector/Scalar by level
parity. One contiguous 576 KiB store per level ([128, 2304] tile,
4.5 KiB runs per partition).
"""

import numpy as np
import ml_dtypes

import concourse.mybir as mybir
from concourse import bacc, bass
from concourse.tile import TileContext
from concourse.bass_utils import run_bass_kernel_spmd

B, C, N, K = 4, 1, 1024, 16
NCORES = 8
K3 = 3 * K             # stacked contraction partitions
PACK = 2304            # packed free dim per level (1024+896+384)

F32 = mybir.dt.float32
BF16 = mybir.dt.bfloat16
BF16_NP = ml_dtypes.bfloat16

# pieces per core-half: (block_row, col0, col1); identical shape lists
# (512,512,512,384,256,128) on both halves. Pack order pairs them into
# segments of 1024, 896 and 384 columns.
PIECES = [
    [(0, 0, 512), (0, 512, 1024), (4, 512, 1024),
     (1, 640, 1024), (2, 768, 1024), (3, 896, 1024)],
    [(1, 128, 640), (2, 256, 768), (3, 384, 896),
     (5, 640, 1024), (6, 768, 1024), (7, 896, 1024)],
]
# pieces (0,1) -> psum tile A [128,1024]; (2,3) -> B [128,896]; (4,5) -> C
SEG = [(0, 2, 1024), (2, 4, 896), (4, 6, 384)]  # (piece lo, hi, width)

_nc_cache = None


def _build():
    nc = bacc.Bacc(None, target_bir_lowering=False)
    # tr: (A,B,A) k-stacking, rhs columns host-packed per piece so the
    # SPMD program uses identical packed offsets on both core-halves
    tr_d = nc.declare_dram_parameter("tr", [K3, PACK], BF16, isOutput=False)
    # tlb: (A,A,B) k-stacking, 6 x 128 piece rows (lhsT side)
    tlb_d = nc.declare_dram_parameter("tlb", [K3, 768], BF16, isOutput=False)
    out_d = nc.declare_dram_parameter("out", [K, 128, PACK], BF16,
                                      isOutput=True)

    with TileContext(nc) as tc:
        with (
            tc.tile_pool(name="vpool", bufs=1) as vpool,
            tc.tile_pool(name="stage", bufs=6) as stage,
            tc.tile_pool(name="psum", bufs=1, space=bass.MemorySpace.PSUM) as psum,
        ):
            tr = vpool.tile([K3, PACK], BF16)
            tlb = vpool.tile([K3, 768], BF16)
            tr0 = vpool.tile([9, PACK], BF16)
            tlb0 = vpool.tile([9, 768], BF16)
            # early slices cover levels 0-2 (kk<=9); big loads follow
            nc.sync.dma_start(out=tlb0[:], in_=tlb_d[:9, :])
            nc.scalar.dma_start(out=tr0[:], in_=tr_d[:9, :])
            nc.sync.dma_start(out=tlb[:], in_=tlb_d[:])
            nc.scalar.dma_start(out=tr[:], in_=tr_d[:])

            sizes = [512, 512, 512, 384, 256, 128]
            poffs = [0, 512, 1024, 1536, 1920, 2176]
            bufs = {0: 2, 1: 1, 2: 2}  # 2*2+1*2+2*1 = 7 PSUM banks
            for l in range(K):
                kk = 3 * (l + 1)
                lhs_t, rhs_t = (tlb0, tr0) if l <= 2 else (tlb, tr)
                st = stage.tile([128, PACK], BF16, tag="st", name=f"st{l}")
                off = 0
                for s, (plo, phi, w) in enumerate(SEG):
                    ps = psum.tile([128, w], F32, tag=f"ps{s}",
                                   bufs=bufs[s], name=f"ps{l}_{s}")
                    for p in range(plo, phi):
                        o = poffs[p] - off
                        nc.tensor.matmul(
                            ps[:, o:o + sizes[p]],
                            lhsT=lhs_t[:kk, 128 * p:128 * (p + 1)],
                            rhs=rhs_t[:kk, poffs[p]:poffs[p] + sizes[p]],
                            start=True,
                            stop=True,
                        )
                    # cast copy PSUM->SBUF; alternate engines by level
                    if (l + s) % 2 == 0:
                        nc.scalar.copy(st[:, off:off + w], ps[:])
                    else:
                        nc.vector.tensor_copy(st[:, off:off + w], ps[:])
                    if l == 0:  # ramp: store per segment
                        nc.sync.dma_start(
                            out=out_d[0, :, off:off + w],
                            in_=st[:, off:off + w],
                        )
                    off += w
                if l > 0:
                    nc.sync.dma_start(out=out_d[l], in_=st[:])

    nc.compile()
    return nc


def _get_nc():
    global _nc_cache
    if _nc_cache is None:
        _nc_cache = _build()
    return _nc_cache


def _prepare_in_maps(evecs: np.ndarray) -> list[dict]:
    poffs = [0, 512, 1024, 1536, 1920, 2176]
    in_maps = []
    for c in range(NCORES):
        b, h = divmod(c, 2)
        vt = np.ascontiguousarray(evecs[b, 0].T, dtype=np.float32)  # [K, N]
        a32 = vt.astype(BF16_NP).astype(np.float32)
        a = a32.astype(BF16_NP)                       # hi part
        bb = (vt - a32).astype(BF16_NP)               # lo part
        tr = np.empty((K3, PACK), dtype=BF16_NP)
        tlb = np.empty((K3, 768), dtype=BF16_NP)
        for p, (i, c0, c1) in enumerate(PIECES[h]):
            cs = slice(poffs[p], poffs[p] + (c1 - c0))
            tr[0::3, cs] = a[:, c0:c1]
            tr[1::3, cs] = bb[:, c0:c1]
            tr[2::3, cs] = a[:, c0:c1]
            rs = slice(128 * i, 128 * (i + 1))
            tlb[0::3, 128 * p:128 * (p + 1)] = a[:, rs]
            tlb[1::3, 128 * p:128 * (p + 1)] = a[:, rs]
            tlb[2::3, 128 * p:128 * (p + 1)] = bb[:, rs]
        in_maps.append({"tr": tr, "tlb": tlb})
    return in_maps


def _assemble(results: list[dict]) -> np.ndarray:
    out = np.empty((B, K, N, N), dtype=np.float32)
    # packed free-dim offset of each piece
    offs = np.cumsum([0, 512, 512, 512, 384, 256])
    for b in range(B):
        for h in range(2):
            r = results[2 * b + h]["out"].astype(np.float32)  # [K,128,PACK]
            for p, (i, c0, c1) in enumerate(PIECES[h]):
                out[b, :, 128 * i:128 * (i + 1), c0:c1] = \
                    r[:, :, offs[p]:offs[p] + (c1 - c0)]
        # mirror the strictly-lower blocks from the computed upper ones
        for i in range(1, 8):
            out[b, :, 128 * i:128 * (i + 1), :128 * i] = np.swapaxes(
                out[b, :, :128 * i, 128 * i:128 * (i + 1)], -1, -2
            )
    return out.reshape(B, K * C, N, N)


def kernel(evecs) -> np.ndarray:
    evecs = np.asarray(evecs, dtype=np.float32)
    assert evecs.shape == (B, C, N, K), evecs.shape
    nc = _get_nc()
    in_maps = _prepare_in_maps(evecs)
    last_err = None
    for _attempt in range(3):
        try:
            r = run_bass_kernel_spmd(nc, in_maps, list(range(NCORES)))
            return _assemble(r.results)
        except Exception as e:  # transient NRT/device hiccups: retry
            last_err = e
    raise last_err
